# revision 38
# baseline (speedup 1.0000x reference)
"""Trainium2 Bass kernel for nn_CompatibleLearningLoss (MoCo-style queue contrastive loss).

Splits the loss  -mean_i( sum_j mask*w*(s_j - m - logZ) / M_i )  into
  * a sparse part  A = sum_j mask*w*s_j  and  W = sum_j mask*w : the label
    mask has ~Q/C = 4 positives per row, so these are ~512 short dot products
    — computed EXACTLY on the host in float64;
  * a dense part  m_i = max_j s_j  and  Z_i = sum_j exp(s_j - m_i)  over the
    full queue, for both score matrices
        scores1 = new_e      @ feat_queue.T      [N, Q]
        scores2 = new_logits @ logit_queue.T     [N, Q]
    — the only work that actually needs the 1 GB queue streamed, done on
    device with the queue dimension sharded across 8 NeuronCores.

Each core streams its 4096-row queue shard once and emits per-row partial
softmax stats (chunk maxes + exp-sums vs chunk max) into a [128, 128] stats
tile; the host combines partials in float64.

The kernel is jointly HBM- and TensorE-bound, so every matmul operand is
fp8-e4m3 and every matmul runs in DoubleRow perf mode (2 MACs/cycle, pair
dim = two consecutive 128-row contraction blocks).  Feat-path operands are
scaled x16 so their small elements land in e4m3's normal range; the
resulting x256 score scale folds into the stats activation's `scale`.  The
logit stream is split into four 1024-column blocks with separate 2-bank
PSUM accumulators so each block's stats hide under the next block's
DMA/matmul stream (only the last block's stats sit in the tail), and phase A
(feat) runs in four 1024-column quarters ping-ponging 2-bank PSUM tiles so
TensorE never waits on the stats consumers.  Deep SBUF buffering (~8 MB of
logit chunks) keeps the two HWDGE rings streaming across phase and
iteration boundaries.  Marshaling (all host-side): circular queue scatter,
new_embeds normalization, sparse-part evaluation, fp8 quantization, and
pre-transposition so the contraction dim lands on SBUF partitions.
"""

from contextlib import nullcontext

import numpy as np

import concourse.bass as bass
import concourse.tile as tile
from concourse import mybir
from concourse.bass_utils import run_bass_kernel_spmd
from concourse.vector_clock import ScopedClock

N = 128      # batch
D = 512      # embed dim
C = 8192     # logit dim
Q = 32768    # queue length
N_CORES = 8
QS = Q // N_CORES          # 4096 queue rows per core
BSPLIT = 4                 # logit stream column blocks (tail = 1 block's stats)
EPS = 1e-12
FSCALE = 16.0              # feat-path fp8 scaling (featT x16, stationaries /16)

F32 = mybir.dt.float32
F8 = mybir.dt.float8e4
DR = mybir.MatmulPerfMode.DoubleRow

# stats tile column layout (per core, [128, 128] f32)
# cols 0:8      m1 parts   (feat-path chunk maxes, 8 chunks of 512)
# cols 8:16     z1 parts   (feat-path sum exp(s - chunk max))
# cols 32:40    m2 parts   (logit-path chunk maxes, quarter q at 32+2q)
# cols 64:72    z2 parts   (quarter q at 64+2q)
CB2 = C // 256             # 32 DoubleRow contraction chunks (logit path)


def _split_excess_waits(nc: bass.Bass, limit: int = 1) -> None:
    """This walrus build rejects instructions carrying more than one sync wait
    ("Too many sync wait commands").  Tile's sem-assignment freely attaches
    several.  Move excess waits onto same-engine nops inserted right before
    the offending instruction (queue order makes that equivalent)."""
    for f in nc.m.functions:
        for bb in f.blocks:
            insts = bb.instructions
            insertions = []
            for idx, inst in enumerate(insts):
                si = inst.sync_info
                if si is None:
                    continue
                cap = 2 if isinstance(inst, mybir.InstEventSemaphore) else limit
                waits = list(si.on_wait)
                if len(waits) <= cap:
                    continue
                keep = waits[:cap]
                excess = waits[cap:]
                si.on_wait = keep
                nops = []
                for w in excess:
                    nop = mybir.InstNoOp(
                        name=nc.get_next_instruction_name(), ins=[], outs=[]
                    )
                    nop.engine = inst.engine
                    nop.sync_info = mybir.SyncInfo(on_wait=[w], on_update=[])
                    nc.register_instruction(nop, overwrite=True)
                    nops.append(nop)
                insertions.append((idx, nops))
            for idx, nops in reversed(insertions):
                for nop in reversed(nops):
                    bb.instructions.insert(idx, nop)


class PatchedTileContext(tile.TileContext):
    """Work around the 1-sync-wait-per-instruction cap in this walrus build:
    the stock TileContext tail drain carries one wait per outstanding proc,
    which codegen rejects ("Too many sync wait commands").  Split the waits
    across single-wait SP nops instead."""

    def _drain_and_barrier(self, tick_clock, wait_clock):
        drain_inst = self.nc.sync.drain()
        wait_clock.add_sem_waits(
            drain_inst.ins, ScopedClock({None: tick_clock.global_clock})
        )
        si = drain_inst.ins.sync_info
        if si is not None and len(si.on_wait) > 1:
            waits = list(si.on_wait)
            si.on_wait = [waits[0]]
            for w in waits[1:]:
                nop = self.nc.sync.nop(nofuse=True, hint="drain_wait_split")
                nop.ins.sync_info = mybir.SyncInfo(on_wait=[w], on_update=[])
        self.nc.all_engine_barrier()
        assert self.sems is not None
        popped = self.nc._tile_sem_poison_stack.pop()
        assert popped is self._sem_poison
        self.nc.clear_and_free_semaphores(list(self.sems.allocated().values()))
        self.nc.all_engine_barrier()


def _build_program(repeat: int = 1, mode: str = "full",
                   bsplit: int = BSPLIT) -> bass.Bass:
    # mode: "full" | "dma" (skip matmuls+stats) | "mm" (skip DMAs) — perf probes
    do_mm = mode != "dma"
    do_dma = mode != "mm"
    do_stats = mode == "full"
    qb_cols = QS // bsplit       # columns per logit block
    qb_ch = qb_cols // 512       # 512-col stat chunks per block
    nc = bass.Bass()

    # DoubleRow moving layouts (pair dim = two consecutive 128-row blocks of
    # the contraction dim):
    # featT[qq, d2, p, r, j] = 16 * feat_queue[qs0 + qq*1024 + j, d2*256 + r*128 + p]
    featT = nc.dram_tensor("featT", [4, 2, 128, 2, 1024], F8,
                           kind="ExternalInput")
    # logitT[qb, c2, p, r, j] = logit_queue[qs0 + qb*qb_cols + j, c2*256 + r*128 + p]
    logitT = nc.dram_tensor("logitT", [bsplit, CB2, 128, 2, qb_cols], F8,
                            kind="ExternalInput")
    # stationary operands pre-arranged host-side as their SBUF image
    # [partition, chunk, batch] so each DMA is one contiguous run per partition
    neT = nc.dram_tensor("neT", [128, D // 128, N], F8, kind="ExternalInput")
    nlT = nc.dram_tensor("nlT", [128, C // 128, N], F8, kind="ExternalInput")
    stats = nc.dram_tensor("stats", [N, 128], F32, kind="ExternalOutput")

    AX = mybir.AxisListType
    OP = mybir.AluOpType
    ACT = mybir.ActivationFunctionType

    with PatchedTileContext(nc) as tc:
        with (
            tc.tile_pool(name="const", bufs=1) as const,
            tc.tile_pool(name="small", bufs=4) as small,
            tc.tile_pool(name="scr", bufs=2) as scrp,
            tc.tile_pool(name="ftp", bufs=8) as ftp,
        ):
            # replicated stationary operands, pre-transposed host-side
            neT_sb = const.tile([128, D // 128, N], F8)
            nc.gpsimd.dma_start(out=neT_sb, in_=neT[:, :, :])
            nlT_sb = const.tile([128, C // 128, N], F8)
            nc.gpsimd.dma_start(out=nlT_sb, in_=nlT[:, :, :])

            out_sb = const.tile([N, 128], F32)
            if not do_stats:
                nc.vector.memset(out_sb, 0.0)

            def stats_block(src, col_m, col_z, nch, scale=1.0):
                """Per-row softmax stats over a [128, nch, 512] block `src` of
                scores scaled by 1/`scale`: raw chunk maxes -> cols [col_m,
                col_m+nch) (host rescales), exp-sums of the true-scale scores
                vs own chunk max -> cols [col_z, col_z+nch)."""
                nc.vector.tensor_reduce(
                    out=out_sb[:, col_m : col_m + nch], in_=src,
                    axis=AX.X, op=OP.max,
                )
                negm = small.tile([128, 8], F32, tag="negm")
                nc.vector.tensor_scalar_mul(
                    out=negm[:, :nch], in0=out_sb[:, col_m : col_m + nch],
                    scalar1=-scale,
                )
                for k in range(nch):
                    escr = scrp.tile([128, 512], F32, tag="escr")
                    nc.scalar.activation(
                        out=escr, in_=src[:, k, :], func=ACT.Exp,
                        bias=negm[:, k : k + 1], scale=scale,
                        accum_out=out_sb[:, col_z + k : col_z + k + 1],
                    )

            loop_cm = tc.For_i(0, repeat, 1) if repeat > 1 else nullcontext()
            with loop_cm:
                # ---- Phase A: feat path, four 1024-column quarters, both
                # operands fp8 x16 with DoubleRow (scores land x256; the 1/256
                # folds into the stats activation scale).  2 PSUM banks per
                # quarter, pool bufs=2: quarter k+1's matmuls run while
                # quarter k's stats read its PSUM directly.
                with tc.tile_pool(name="psum_a", bufs=2, space="PSUM") as psum_a:
                    for qq in range(4):
                        ps1 = psum_a.tile([128, 2, 512], F32, tag="ps1")
                        for d2 in range(2):
                            ft = ftp.tile([128, 2, 1024], F8, tag="ft")
                            dma_eng = nc.sync if d2 % 2 == 0 else nc.scalar
                            if do_dma:
                                dma_eng.dma_start(
                                    out=ft, in_=featT[qq, d2, :, :, :],
                                )
                            elif do_mm:
                                dma_eng.dma_start(
                                    out=ft[:, :, 0:16],
                                    in_=featT[qq, d2, :, :, 0:16],
                                )
                            for qw in range(2):
                                if not do_mm:
                                    break
                                nc.tensor.matmul(
                                    ps1[:, qw, :],
                                    neT_sb[:, 2 * d2 : 2 * d2 + 2, :],
                                    ft[:, :, qw * 512 : (qw + 1) * 512],
                                    start=(d2 == 0), stop=(d2 == 1),
                                    perf_mode=DR,
                                )
                        if do_stats:
                            stats_block(ps1, 2 * qq, 8 + 2 * qq, 2,
                                        scale=1.0 / (FSCALE * FSCALE))

                # ---- Phase B: logit path (fp8 DoubleRow), `bsplit` column
                # blocks with separate PSUM accumulators; block q's stats run
                # under block q+1's DMA/matmul stream, so only the last
                # block's stats sit in the tail.
                with (
                    tc.tile_pool(name="tcp", bufs=8 * bsplit) as tcp,  # ~8 MB
                    tc.tile_pool(name="psum_b", bufs=1, space="PSUM") as psum_b,
                ):
                    psB = [psum_b.tile([128, qb_ch, 512], F32, tag=f"psB{qb}",
                                       name=f"psB{qb}")
                           for qb in range(bsplit)]

                    def b_step(qb, cb):
                        tcb = tcp.tile([128, 2, qb_cols], F8, tag="tcb",
                                       name="tcb")
                        dma_eng = nc.sync if cb % 2 == 0 else nc.scalar
                        if do_dma:
                            dma_eng.dma_start(out=tcb, in_=logitT[qb, cb, :, :, :])
                        elif do_mm:
                            dma_eng.dma_start(
                                out=tcb[:, :, 0:16], in_=logitT[qb, cb, :, :, 0:16]
                            )
                        for qw in range(qb_ch):
                            if not do_mm:
                                break
                            nc.tensor.matmul(
                                psB[qb][:, qw, :],
                                nlT_sb[:, 2 * cb : 2 * cb + 2, :],
                                tcb[:, :, qw * 512 : (qw + 1) * 512],
                                start=(cb == 0), stop=(cb == CB2 - 1),
                                perf_mode=DR,
                            )

                    for qb in range(bsplit):
                        for cb in range(CB2):
                            b_step(qb, cb)
                        if do_stats and qb > 0:
                            # previous block's stats under this stream
                            stats_block(psB[qb - 1], 32 + qb_ch * (qb - 1),
                                        64 + qb_ch * (qb - 1), qb_ch)
                    if do_stats:
                        stats_block(psB[bsplit - 1], 32 + qb_ch * (bsplit - 1),
                                    64 + qb_ch * (bsplit - 1), qb_ch)

            nc.sync.dma_start(out=stats[:, :], in_=out_sb)

    _split_excess_waits(nc)
    return nc


_PROGRAM: bass.Bass | None = None
LAST_RESULTS = None  # BassKernelResults of the most recent run (for profiling)


def _get_program() -> bass.Bass:
    global _PROGRAM
    if _PROGRAM is None:
        _PROGRAM = _build_program()
    return _PROGRAM


def _to_f8(t):
    """torch f32 tensor -> numpy ml_dtypes.float8_e4m3 view, same shape.
    torch's e4m3fn and TRN/ml_dtypes e4m3 agree bit-for-bit for |x| <= 240;
    all tensors quantized here are well inside that."""
    import ml_dtypes
    import torch

    return (
        t.to(torch.float8_e4m3fn).view(torch.int8).numpy()
        .view(ml_dtypes.float8_e4m3)
    )


def host_prep(old_embeds, old_logits, new_embeds, new_logits, labels,
              feat_queue, logit_queue, queue_labels, header,
              bsplit: int = BSPLIT):
    """Scatter + normalize + quantize + pre-transpose on host; evaluates the
    sparse masked sums exactly in float64.  Returns per-core in_maps and
    (W, A1, A2, M) host vectors."""
    import torch

    old_embeds = np.asarray(old_embeds, dtype=np.float32)
    old_logits = np.asarray(old_logits, dtype=np.float32)
    new_embeds = np.asarray(new_embeds, dtype=np.float32)
    new_logits = np.asarray(new_logits, dtype=np.float32)
    feat_queue = np.array(feat_queue, dtype=np.float32)   # copies (scattered below)
    logit_queue = np.array(logit_queue, dtype=np.float32)
    labels_np = np.asarray(labels).astype(np.int64)
    queue_labels_np = np.asarray(queue_labels).astype(np.int64)
    hdr = int(np.asarray(header))

    n = old_embeds.shape[0]
    q = feat_queue.shape[0]
    assert (n, q) == (N, Q)

    # circular queue scatter
    idx = (hdr + np.arange(n)) % q
    feat_queue[idx] = old_embeds
    logit_queue[idx] = old_logits
    queue_labels_np[idx] = labels_np

    # normalize new_embeds (f64 intermediate, f32 result)
    ne64 = new_embeds.astype(np.float64)
    norm = np.sqrt((ne64 * ne64).sum(axis=1, keepdims=True))
    new_e = (ne64 / np.maximum(norm, EPS)).astype(np.float32)

    # ---- sparse part (exact, host): the label mask has ~Q/C positives per
    # row, so  W = sum_j mask*w,  A = sum_j mask*w*s  are ~512 short dot
    # products in float64.
    mask = queue_labels_np[None, :] == labels_np[:, None]
    M = mask.sum(axis=1).astype(np.float64)               # [N], >= 1 by construction
    rows, cols = np.nonzero(mask)
    w_v = 0.5 * ((old_embeds[rows].astype(np.float64)
                  * feat_queue[cols].astype(np.float64)).sum(axis=1) + 1.0)
    s1_v = (new_e[rows].astype(np.float64)
            * feat_queue[cols].astype(np.float64)).sum(axis=1)
    s2_v = (new_logits[rows].astype(np.float64)
            * logit_queue[cols].astype(np.float64)).sum(axis=1)
    W = np.bincount(rows, weights=w_v, minlength=N)
    A1 = np.bincount(rows, weights=w_v * s1_v, minlength=N)
    A2 = np.bincount(rows, weights=w_v * s2_v, minlength=N)

    def _sbuf_image(aT):
        # [K, N] -> [128, K//128, N] partition-major SBUF image
        k = aT.shape[0]
        return np.ascontiguousarray(
            aT.reshape(k // 128, 128, aT.shape[1]).transpose(1, 0, 2)
        )

    import ml_dtypes

    # feat-path operands scaled x16 so elements (~N(0, 1/512)) land in
    # e4m3's normal range; scores come out x256, rescaled in the stats.
    neT_t = torch.from_numpy(new_e * FSCALE).t().contiguous()  # [D, N]
    neT = _sbuf_image(_to_f8(neT_t))                          # [128, 4, N] fp8
    nlT_t = torch.from_numpy(new_logits).t().contiguous()     # [C, N]
    nlT = _sbuf_image(_to_f8(nlT_t))                          # [128, 64, N] fp8

    fq_t = torch.from_numpy(feat_queue)
    lq8 = torch.from_numpy(logit_queue).to(torch.float8_e4m3fn)  # [Q, C]

    in_maps = []
    for d in range(N_CORES):
        sl = slice(d * QS, (d + 1) * QS)
        # feat shard -> DoubleRow moving layout [4, 2, 128, 2, 1024]:
        # [qq, d2, p, r, j] = 16 * feat_queue[qs0 + qq*1024 + j, d2*256 + r*128 + p]
        f8 = (fq_t[sl].t() * FSCALE).to(torch.float8_e4m3fn)     # [D, QS]
        featT = (f8.view(torch.int8).reshape(2, 2, 128, 4, 1024)
                 .permute(3, 0, 2, 1, 4).contiguous().numpy()
                 .view(ml_dtypes.float8_e4m3))
        # logit shard -> DoubleRow moving layout [bsplit, CB2, 128, 2, cols]:
        # [qb, c2, p, r, j] = logit_queue[qs0 + qb*cols + j, c2*256 + r*128 + p]
        lsh = lq8[sl].view(torch.int8).t().contiguous()          # [C, QS] i8
        ldr = (lsh.reshape(CB2, 2, 128, bsplit, QS // bsplit)
               .permute(3, 0, 2, 1, 4)
               .contiguous().numpy().view(ml_dtypes.float8_e4m3))
        in_maps.append({
            "featT": featT,
            "logitT": ldr,
            "neT": neT,
            "nlT": nlT,
        })
    return in_maps, (W, A1, A2, M)


def combine_stats(parts: np.ndarray, host_sums):
    """parts: [n_cores, 128, 128] f32 stats tiles + exact host sparse sums
    -> (l1, l2) f32 scalars."""
    W, A1, A2, M = host_sums
    parts = parts.astype(np.float64)
    m1p = parts[:, :, 0:8]
    z1p = parts[:, :, 8:16]
    m2p = parts[:, :, 32:40]
    z2p = parts[:, :, 64:72]

    m1p = m1p / (FSCALE * FSCALE)   # device stores raw x256 feat maxes
    m1 = m1p.max(axis=(0, 2))
    m2 = m2p.max(axis=(0, 2))
    Z1 = (z1p * np.exp(m1p - m1[None, :, None])).sum(axis=(0, 2))
    Z2 = (z2p * np.exp(m2p - m2[None, :, None])).sum(axis=(0, 2))

    # sum_j maskw * log_prob = A - (m + log Z) * W ; divide by count, mean, negate
    l1 = -np.mean((A1 - (m1 + np.log(Z1)) * W) / M)
    l2 = -np.mean((A2 - (m2 + np.log(Z2)) * W) / M)
    return (np.float32(l1), np.float32(l2))


def kernel(old_embeds, old_logits, new_embeds, new_logits, labels,
           feat_queue, logit_queue, queue_labels, header):
    global LAST_RESULTS
    in_maps, host_sums = host_prep(
        old_embeds, old_logits, new_embeds, new_logits, labels,
        feat_queue, logit_queue, queue_labels, header,
    )
    nc = _get_program()
    LAST_RESULTS = run_bass_kernel_spmd(nc, in_maps, list(range(N_CORES)))
    parts = np.stack([LAST_RESULTS.results[d]["stats"] for d in range(N_CORES)])
    return combine_stats(parts, host_sums)
